# revision 40
# baseline (speedup 1.0000x reference)
import os
import sys

sys.path.insert(0, '/opt/trn_rl_repo')
import numpy as np
import ml_dtypes

BF16 = ml_dtypes.bfloat16

N_CORES = 8
SA = 12500         # authors per core shard
SP = 18750         # papers per core shard
HL = 6144          # af2 lo-half rows per core (= 12 superblocks of 512)
SB = 512           # psum superblock (dst columns per accumulation block)
GB = int(os.environ.get('KERNEL_GB', '8'))   # tiles per dma_gather (1024-idx SWDGE ring limit)

# af2 table row chunks (half-major layout, int16-safe)
CH_A = [0, 24576, 49152, 74576, 100000]
# paper table row chunks
CH_P = [0, 30000, 60000, 90000, 120000, 150000]


def _ceil(a, b):
    return -(-a // b)


def _wrap_idx(arr):
    """[n*128] int16 -> [128, n*8]: idx i at (i%16, i//16), replicated x8."""
    n = arr.shape[0]
    w16 = arr.reshape(n // 16, 16).T
    return np.tile(w16, (8, 1))


def _build_direction(src_rows, dst, S, chunks, W, OVW, BIAS):
    """Bucket edges for one direction.

    Regular buckets: (window m of width W, chunk k), slot count = floor(mean
    over cores / 128 + BIAS).  Per-core overflow pooled per (OVW-window h,
    chunk k) into width-OVW tiles sized by cross-core max.

    Tile order: for j, for k: [regular tiles of windows in j] + [overflow
    tiles of (h in j, k)]; each (j,k) run split into batches of <= GB tiles.
    Returns (meta, idx_staged[per core], dl_staged[per core]).
    """
    E = src_rows.shape[0]
    nch = len(chunks) - 1
    nj = _ceil(S, SB)
    nm = _ceil(S, W)
    nh = _ceil(S, OVW)
    wpj = SB // W   # regular windows per superblock
    hpj = SB // OVW  # overflow windows per superblock

    core = (dst // S).astype(np.int64)
    drel = dst - core * S
    k_arr = np.searchsorted(chunks, src_rows, side='right') - 1
    src_loc = (src_rows - np.asarray(chunks)[k_arr]).astype(np.int16)
    m_arr = drel // W

    # counts per (core, m, k)
    bidx = (core * nm + m_arr) * nch + k_arr
    cnt = np.bincount(bidx, minlength=N_CORES * nm * nch).reshape(N_CORES, nm, nch)
    slots = np.floor(cnt.mean(axis=0) / 128.0 + BIAS).astype(np.int64)  # [nm, nch]
    cap = slots * 128
    ovcnt = np.maximum(0, cnt - cap[None])          # [NC, nm, nch]
    ovkh = np.zeros((N_CORES, nh, nch), np.int64)
    m2h = (np.arange(nm) * W) // OVW
    for c in range(N_CORES):
        np.add.at(ovkh[c], (m2h[:, None], np.arange(nch)[None, :]), ovcnt[c])
    ovslots = _ceil(ovkh, 128).max(axis=0)          # [nh, nch]

    # tile table in emission order
    tile_w, tile_anchor = [], []
    reg_tile_start = np.zeros((nm, nch), np.int64)
    ov_tile_start = np.zeros((nh, nch), np.int64)
    runs = []  # runs[j][k] = list of batches [(col0, nt, tile0)]
    col = 0
    t = 0
    for j in range(nj):
        runs_j = []
        for k in range(nch):
            run_t0 = t
            for m in range(j * wpj, min((j + 1) * wpj, nm)):
                reg_tile_start[m, k] = t
                for _ in range(int(slots[m, k])):
                    tile_w.append(W); tile_anchor.append((m % wpj) * W)
                    t += 1
            for h in range(j * hpj, min((j + 1) * hpj, nh)):
                ov_tile_start[h, k] = t
                for _ in range(int(ovslots[h, k])):
                    tile_w.append(OVW); tile_anchor.append((h % hpj) * OVW)
                    t += 1
            nrun = t - run_t0
            batches = []
            for b0 in range(0, nrun, GB):
                nt = min(GB, nrun - b0)
                batches.append((col, nt, run_t0 + b0))
                col += nt * 8
            col += nrun  # dl columns (bf16 bit-cast in the i16 slab)
            runs_j.append(batches)
        runs.append(runs_j)
    ntiles = t
    ncols = col

    tile_w = np.array(tile_w); tile_anchor = np.array(tile_anchor)

    # --- per-core lane assignment ---
    idx_staged, dl_staged = [], []
    # rank of each edge within its (core, m, k) group
    order = np.argsort(bidx, kind='stable')
    bs = bidx[order]
    grp_start = np.searchsorted(bs, bs, side='left')
    rank = np.empty(E, np.int64)
    rank[order] = np.arange(E) - grp_start

    cap_e = cap[m_arr, k_arr]
    is_reg = rank < cap_e
    # overflow rank within (core, h, k)
    h_arr = drel // OVW
    ovb = (core * nh + h_arr) * nch + k_arr
    ov_sel = ~is_reg
    ovb_sel = ovb[ov_sel]
    order2 = np.argsort(ovb_sel, kind='stable')
    obs = ovb_sel[order2]
    grp2 = np.searchsorted(obs, obs, side='left')
    rank2 = np.empty(ovb_sel.shape[0], np.int64)
    rank2[order2] = np.arange(ovb_sel.shape[0]) - grp2

    gtile = np.empty(E, np.int64)
    lane = np.empty(E, np.int64)
    gtile[is_reg] = reg_tile_start[m_arr[is_reg], k_arr[is_reg]] + rank[is_reg] // 128
    lane[is_reg] = rank[is_reg] % 128
    gtile[ov_sel] = ov_tile_start[h_arr[ov_sel], k_arr[ov_sel]] + rank2 // 128
    lane[ov_sel] = rank2 % 128
    dlv = np.where(is_reg, drel - (m_arr * W), drel - (h_arr * OVW)).astype(np.float32)

    for c in range(N_CORES):
        sel = core == c
        idx_flat = np.zeros(ntiles * 128, np.int16)
        idx_flat[gtile[sel] * 128 + lane[sel]] = src_loc[sel]
        dl = np.full((128, ntiles), -1.0, np.float32)
        dl[lane[sel], gtile[sel]] = dlv[sel]

        idx_cols = np.zeros((128, ncols), np.int16)
        per_tile = idx_flat.reshape(ntiles, 128)
        for j in range(nj):
            for k in range(nch):
                if not runs[j][k]:
                    continue
                for (c0, nt, t0) in runs[j][k]:
                    idx_cols[:, c0:c0 + nt * 8] = _wrap_idx(
                        per_tile[t0:t0 + nt].reshape(-1))
                rc0, _, rt0 = runs[j][k][0]
                ntot = sum(b[1] for b in runs[j][k])
                idx_cols[:, rc0 + ntot * 8: rc0 + ntot * 8 + ntot] = (
                    dl[:, rt0:rt0 + ntot].astype(BF16).view(np.int16))
        idx_staged.append(idx_cols)
        dl_staged.append(dl)

    meta = dict(S=S, nch=nch, nj=nj, ntiles=ntiles, ncols=ncols, runs=runs,
                tile_w=tile_w, tile_anchor=tile_anchor)
    return meta, idx_staged, dl_staged


def _af2row(u):
    c = u // SA
    r = u - c * SA
    return np.where(r < HL, c * HL + r, 8 * HL + c * (SA - HL) + (r - HL))


def kernel(author_features, edge_author, edge_paper, paper_emb, Wproj, bproj,
           W1l_ap, b1_ap, W1r_ap, W1l_pa, b1_pa, W1r_pa,
           W2l_ap, b2_ap, W2r_ap, W2l_pa, b2_pa, W2r_pa,
           Wl1, bl1, Wl2, bl2):
    import concourse.bass as bass
    import concourse.tile as tile
    from concourse import bacc, mybir
    from concourse.bass_utils import run_bass_kernel_spmd
    from concourse.library_config import mlp

    f32 = mybir.dt.float32
    bf16 = mybir.dt.bfloat16
    i16 = mybir.dt.int16
    AF = mybir.ActivationFunctionType

    author_features = np.asarray(author_features, np.float32)
    paper_emb = np.asarray(paper_emb, np.float32)
    ea = np.asarray(edge_author, np.int64)
    ep = np.asarray(edge_paper, np.int64)

    NA, FIN = author_features.shape
    NP_, H = paper_emb.shape
    assert H == 64 and FIN == 128 and NA == 100000 and NP_ == 150000

    # ---- host prep ----
    mAP, idxAP, dlAP = _build_direction(_af2row(ea), ep, SP, CH_A,
                                        W=64, OVW=256, BIAS=0.5)
    mPA, idxPA, dlPA = _build_direction(ep, ea, SA, CH_P,
                                        W=64, OVW=256, BIAS=0.25)

    afT_cores, pT65_cores, invA_cores, invP_cores = [], [], [], []
    for c in range(N_CORES):
        blk = author_features[c * SA: (c + 1) * SA]
        pad = np.zeros((SA, FIN), np.float32)
        pad[:blk.shape[0]] = blk
        afT_cores.append(np.ascontiguousarray(pad.T).astype(BF16))
        pblk = paper_emb[c * SP: (c + 1) * SP]
        t65 = np.zeros((65, SP), np.float32)
        t65[:64, :pblk.shape[0]] = pblk.T
        t65[64, :] = 1.0
        pT65_cores.append(t65.astype(BF16))
        cntA = np.bincount(ea[(ea >= c * SA) & (ea < (c + 1) * SA)] - c * SA,
                           minlength=SA).astype(np.float32)
        invA_cores.append(np.tile((1.0 / np.maximum(cntA, 1.0))[None, :], (64, 1)).astype(BF16))
        cntP = np.bincount(ep[(ep >= c * SP) & (ep < (c + 1) * SP)] - c * SP,
                           minlength=SP).astype(np.float32)
        invP_cores.append(np.tile((1.0 / np.maximum(cntP, 1.0))[None, :], (64, 1)).astype(BF16))

    p_rm = np.zeros((NP_, 128), np.float32)
    p_rm[:, :64] = paper_emb
    p_rm = p_rm.astype(BF16)

    iota512 = np.tile(np.arange(512, dtype=np.float32)[None, :], (128, 1)).astype(BF16)
    ident128 = np.eye(128, dtype=np.float32).astype(BF16)

    wb = lambda x: np.asarray(x, np.float32).astype(BF16)
    stack65 = lambda Wr, b: np.vstack([np.asarray(Wr, np.float32),
                                       np.asarray(b, np.float32)[None]]).astype(BF16)
    w_vals = {
        'wproj': wb(Wproj), 'w1l_pa': wb(W1l_pa), 'w1rb_pa': stack65(W1r_pa, b1_pa),
        'w1l_ap': wb(W1l_ap), 'w1rb_ap': stack65(W1r_ap, b1_ap),
        'w2l_ap': wb(W2l_ap), 'w2rb_ap': stack65(W2r_ap, b2_ap),
        'wl1': wb(Wl1), 'wl2c': wb(np.asarray(Wl2, np.float32).reshape(64, 1)),
    }
    wf_vals = {
        'bl1c': np.asarray(bl1, np.float32).reshape(64, 1),
        'bl2c': np.asarray(bl2, np.float32).reshape(1, 1),
        'bprojc': np.asarray(bproj, np.float32).reshape(64, 1),
        'bprojrep': np.tile(np.asarray(bproj, np.float32).reshape(1, 64), (128, 1)),
    }

    # ---- build program ----
    nc = bacc.Bacc("TRN2", target_bir_lowering=False, debug=False,
                   num_devices=N_CORES, dynamic_dma_scratch_size=16384)

    afT_h = nc.dram_tensor("afT", [128, SA], bf16, kind="ExternalInput")
    p_rm_h = nc.dram_tensor("p_rm", [NP_, 128], bf16, kind="ExternalInput")
    pT65_h = nc.dram_tensor("pT65", [65, SP], bf16, kind="ExternalInput")
    invA_h = nc.dram_tensor("invA", [64, SA], bf16, kind="ExternalInput")
    invP_h = nc.dram_tensor("invP", [64, SP], bf16, kind="ExternalInput")
    idxAP_h = nc.dram_tensor("idxAP", list(idxAP[0].shape), i16, kind="ExternalInput")
    idxPA_h = nc.dram_tensor("idxPA", list(idxPA[0].shape), i16, kind="ExternalInput")
    iota_h = nc.dram_tensor("iota512", [128, 512], bf16, kind="ExternalInput")
    ident_h = nc.dram_tensor("ident128", [128, 128], bf16, kind="ExternalInput")
    wh = {n: nc.dram_tensor(n, list(v.shape), bf16, kind="ExternalInput")
          for n, v in w_vals.items()}
    wfh = {n: nc.dram_tensor(n, list(v.shape), f32, kind="ExternalInput")
           for n, v in wf_vals.items()}
    out_h = nc.dram_tensor("out", [1, SP], f32, kind="ExternalOutput")

    af2_in = nc.dram_tensor("af2_in", [SA, 128], bf16)
    af2_lo = nc.dram_tensor("af2_lo", [8 * HL, 128], bf16, addr_space="Shared")
    af2_hi = nc.dram_tensor("af2_hi", [8 * (SA - HL), 128], bf16, addr_space="Shared")
    part_h = nc.dram_tensor("part", [128, SP], bf16)

    rg = [list(range(N_CORES))]
    STAGE = int(os.environ.get("KERNEL_STAGE", "9"))

    with tile.TileContext(nc) as tc:
        import contextlib
        with contextlib.ExitStack() as ctx:
            const = ctx.enter_context(tc.tile_pool(name="const", bufs=1))
            msg_p = ctx.enter_context(tc.tile_pool(name="msg", bufs=8))
            idx_p = ctx.enter_context(tc.tile_pool(name="idx", bufs=4))
            oh_p = ctx.enter_context(tc.tile_pool(name="oh", bufs=4))
            aft_p = ctx.enter_context(tc.tile_pool(name="aft", bufs=2))
            inv_p = ctx.enter_context(tc.tile_pool(name="inv", bufs=2))
            mean_p = ctx.enter_context(tc.tile_pool(name="mean", bufs=4))
            sml_p = ctx.enter_context(tc.tile_pool(name="sml", bufs=4))
            big_p = ctx.enter_context(tc.tile_pool(name="big", bufs=4))
            psM_p = ctx.enter_context(tc.tile_pool(name="psM", bufs=2, space="PSUM"))
            psE_p = ctx.enter_context(tc.tile_pool(name="psE", bufs=3, space="PSUM"))
            psR_p = ctx.enter_context(tc.tile_pool(name="psR", bufs=2, space="PSUM"))

            nc.gpsimd.load_library(mlp)
            reg_cache = {}

            def nidreg(v):
                if v not in reg_cache:
                    reg_cache[v] = nc.gpsimd.to_reg(v)
                return reg_cache[v]

            wt = {}
            for n, v in w_vals.items():
                ti = const.tile(list(v.shape), bf16, tag=f"w_{n}")
                nc.sync.dma_start(ti[:], wh[n][:])
                wt[n] = ti
            for n, v in wf_vals.items():
                ti = const.tile(list(v.shape), f32, tag=f"w_{n}")
                nc.sync.dma_start(ti[:], wfh[n][:])
                wt[n] = ti

            iota_f = const.tile([128, 512], bf16, tag="iota_f")
            nc.sync.dma_start(iota_f[:], iota_h[:])
            ident_t = const.tile([128, 128], bf16, tag="ident_t")
            nc.sync.dma_start(ident_t[:], ident_h[:])
            zc2 = const.tile([1, 128], bf16, tag="zc2")
            nc.vector.memset(zc2[:], 0.0)
            zr = const.tile([1, SB], bf16, tag="zr")
            nc.vector.memset(zr[:], 0.0)

            aT65 = const.tile([65, SA], bf16, tag="aT65")
            nc.vector.memset(aT65[64:65, :], 1.0)
            pT65 = const.tile([65, SP], bf16, tag="pT65")
            nc.sync.dma_start(pT65[:], pT65_h[:])

            # ---- stage A: projection (one superblock; interleaved into PA) ----
            def emit_proj(jp):
                c0 = jp * SB
                C = min(SB, SA - c0)
                afT_t = aft_p.tile([128, SB], bf16, tag="afT", name=f"afT_{jp}")
                nc.sync.dma_start(afT_t[:, :C], afT_h[:, c0:c0 + C])
                psE = psE_p.tile([64, SB], f32, tag="psE", name=f"psP_{jp}")
                nc.tensor.matmul(psE[:, :C], wt['wproj'][:], afT_t[:, :C],
                                 start=True, stop=True)
                nc.scalar.activation(aT65[0:64, c0:c0 + C], psE[:, :C],
                                     AF.Identity, bias=wt['bprojc'][:])
                nb = _ceil(C, 128)
                o4 = sml_p.tile([128, nb, 64], bf16, tag="o4pr", name=f"o4pr_{jp}")
                for s in range(0, C, 128):
                    ws = min(128, C - s)
                    psR = psR_p.tile([128, 64], f32, tag="psR", name=f"psRp_{jp}_{s}")
                    nc.tensor.matmul(psR[:ws, :], afT_t[:, s:s + ws],
                                     wt['wproj'][:], start=True, stop=True)
                    nc.vector.tensor_tensor(out=o4[:ws, s // 128, :],
                                            in0=psR[:ws, :],
                                            in1=wt['bprojrep'][:ws, :],
                                            op=mybir.AluOpType.add)
                if C == SB:
                    _a = af2_in[:]
                    dst = bass.AP(_a.tensor, _a.offset + c0 * 128,
                                  [[128, 128], [128 * 128, nb], [1, 64]])
                    nc.scalar.dma_start(dst, o4[:])
                else:
                    for s in range(0, C, 128):
                        ws = min(128, C - s)
                        nc.scalar.dma_start(af2_in[c0 + s:c0 + s + ws, 0:64],
                                            o4[:ws, s // 128, :])

            if STAGE >= 1:
                for jp in range(_ceil(SA, SB)):
                    emit_proj(jp)

            # ---- shared conv emitter ----
            def emit_pass(meta, idx_h, src_tabs, kfilter,
                          epilogue, after_j=None, pre_j=None, pre_acc=None):
                """src_tabs[k] = (dram_handle, row0, row1).

                PSUM acc is [128, SB]: rows 0:64 aggregate msg cols 0:64,
                rows 64:128 aggregate msg cols 64:128 (one matmul per tile).
                """
                nch, nj = meta['nch'], meta['nj']
                runs = meta['runs']
                tw, ta = meta['tile_w'], meta['tile_anchor']
                for j in range(nj):
                    if pre_j is not None:
                        pre_j(j)
                    ps = psM_p.tile([128, SB], f32, tag="acc", name=f"acc_{j}")
                    nc.tensor.matmul(ps[:], zc2[:], zr[:],
                                     start=True, stop=False)
                    if pre_acc is not None:
                        pre_acc(j, j * SB, min(SB, meta['S'] - j * SB), ps)
                    runs_j = [(k, runs[j][k]) for k in sorted(kfilter)
                              if runs[j][k]]
                    if runs_j:
                        slab0 = runs_j[0][1][0][0]
                        lb = runs_j[-1][1]
                        slab_end = lb[0][0] + sum(b[1] for b in lb) * 9
                        slab = idx_p.tile([128, slab_end - slab0], i16,
                                          tag="ig", name=f"ig_{j}")
                        nc.sync.dma_start(slab[:], idx_h[:, slab0:slab_end])
                    for k, batches in runs_j:
                        c0 = batches[0][0]
                        t0 = batches[0][2]
                        ntot = sum(b[1] for b in batches)
                        so = c0 - slab0
                        dlt = slab[:, so + ntot * 8:so + ntot * 9].bitcast(bf16)
                        # one-hots: contiguous segments of equal width
                        oh_of = {}
                        seg0 = 0
                        while seg0 < ntot:
                            wseg = int(tw[t0 + seg0])
                            seg1 = seg0
                            while seg1 < ntot and int(tw[t0 + seg1]) == wseg:
                                seg1 += 1
                            ns = seg1 - seg0
                            oh = oh_p.tile([128, ns, wseg], bf16)
                            in0 = dlt[:, seg0:seg1].to_broadcast([128, ns, wseg])

                            _i = iota_f[:, :wseg]
                            in1 = bass.AP(_i.tensor, _i.offset,
                                          [list(_i.ap[0]), [0, ns], list(_i.ap[1])])
                            nc.vector.tensor_tensor(out=oh[:], in0=in0, in1=in1,
                                                    op=mybir.AluOpType.is_equal)
                            for i in range(seg0, seg1):
                                oh_of[i] = (oh, i - seg0, wseg)
                            seg0 = seg1
                        tab, r0, r1 = src_tabs[k]
                        for (cb, nt, tb) in batches:
                            msg = msg_p.tile([128, nt, 128], bf16, tag="msg")
                            nc.gpsimd.dma_gather(
                                msg[:], tab[r0:r1, :],
                                slab[:, so + cb - c0:so + cb - c0 + nt * 8],
                                nt * 128, nidreg(nt * 128), 128)
                            for s in range(nt):
                                ti = tb + s
                                oh, oi, wseg = oh_of[ti - t0]
                                off = int(ta[ti])
                                nc.tensor.matmul(ps[:, off:off + wseg],
                                                 msg[:, s, :], oh[:, oi, :],
                                                 start=False, stop=False)
                    nc.tensor.matmul(ps[:], zc2[:], zr[:],
                                     start=False, stop=True)
                    epilogue(j, j * SB, min(SB, meta['S'] - j * SB), ps)
                    if after_j is not None:
                        after_j(j)

            # ---- stage B: PA conv (papers -> authors) ----
            def epi_pa(j, c0, C, ps):
                invt = inv_p.tile([64, SB], bf16)
                nc.sync.dma_start(invt[:, :C], invA_h[:, c0:c0 + C])
                meanT = mean_p.tile([64, SB], bf16)
                nc.vector.tensor_tensor(out=meanT[:, :C], in0=ps[0:64, :C],
                                        in1=invt[:, :C], op=mybir.AluOpType.mult)
                nb = _ceil(C, 128)
                o4 = sml_p.tile([128, nb, 64], bf16, tag="o4pa", name=f"o4pa_{j}")
                for s in range(0, C, 128):
                    ws = min(128, C - s)
                    psR = psR_p.tile([128, 64], f32, tag="psR")
                    nc.tensor.matmul(psR[:ws, :], meanT[:, s:s + ws],
                                     wt['w1l_pa'][:], start=True, stop=False)
                    nc.tensor.matmul(psR[:ws, :], aT65[:, c0 + s:c0 + s + ws],
                                     wt['w1rb_pa'][:], start=False, stop=True)
                    nc.scalar.activation(o4[:ws, s // 128, :], psR[:ws, :],
                                         AF.Relu)
                if C == SB:
                    _a = af2_in[:]
                    dst = bass.AP(_a.tensor, _a.offset + c0 * 128 + 64,
                                  [[128, 128], [128 * 128, nb], [1, 64]])
                    nc.scalar.dma_start(dst, o4[:])
                else:
                    for s in range(0, C, 128):
                        ws = min(128, C - s)
                        nc.scalar.dma_start(
                            af2_in[c0 + s:c0 + s + ws, 64:128],
                            o4[:ws, s // 128, :])

            def after_pa(j):
                if j == HL // SB - 1 and STAGE >= 3:
                    nc.gpsimd.collective_compute(
                        "AllGather", mybir.AluOpType.bypass, replica_groups=rg,
                        ins=[af2_in[0:HL, :]], outs=[af2_lo[:]])
                if j == mPA['nj'] - 1 and STAGE >= 4:
                    nc.gpsimd.collective_compute(
                        "AllGather", mybir.AluOpType.bypass, replica_groups=rg,
                        ins=[af2_in[HL:SA, :]], outs=[af2_hi[:]])

            pa_tabs = {k: (p_rm_h, CH_P[k], CH_P[k + 1]) for k in range(5)}
            if STAGE >= 2:
                emit_pass(mPA, idxPA_h, pa_tabs, {0, 1, 2, 3, 4},
                          epi_pa, after_j=after_pa)

            # ---- stage C: fused AP sub-pass A (lo chunks) ----
            LOSZ, HISZ = 8 * HL, 8 * (SA - HL)
            ap_tabs = {0: (af2_lo, 0, LOSZ // 2), 1: (af2_lo, LOSZ // 2, LOSZ),
                       2: (af2_hi, 0, HISZ // 2), 3: (af2_hi, HISZ // 2, HISZ)}

            def epi_subA(j, c0, C, ps):
                o = big_p.tile([128, SB], bf16, tag="part", name=f"part_{j}")
                nc.scalar.activation(o[:, :C], ps[:, :C], AF.Identity)
                nc.scalar.dma_start(part_h[:, c0:c0 + C], o[:, :C])

            if STAGE >= 5:
                emit_pass(mAP, idxAP_h, ap_tabs, {0, 1}, epi_subA)

            # ---- stage D: fused AP sub-pass B (hi chunks) + epilogue chain ----
            def pre_subB(j, c0, C, ps):
                pl = big_p.tile([128, SB], bf16, tag="pl", name=f"pl_{j}")
                nc.sync.dma_start(pl[:, :C], part_h[:, c0:c0 + C])
                nc.tensor.matmul(ps[:, :C], ident_t[:], pl[:, :C],
                                 start=False, stop=False)

            def epi_subB(j, c0, C, ps):
                invt = inv_p.tile([64, SB], bf16)
                nc.sync.dma_start(invt[:, :C], invP_h[:, c0:c0 + C])
                means = []
                for r0 in (0, 64):
                    mn = mean_p.tile([64, SB], bf16, tag="mn", name=f"mn_{j}_{r0}")
                    nc.vector.tensor_tensor(out=mn[:, :C], in0=ps[r0:r0 + 64, :C],
                                            in1=invt[:, :C],
                                            op=mybir.AluOpType.mult)
                    means.append(mn)
                p1t = mean_p.tile([65, SB], bf16, tag="p1t")
                nc.vector.memset(p1t[64:65, :], 1.0)
                psE = psE_p.tile([64, SB], f32, tag="psE")
                nc.tensor.matmul(psE[:, :C], wt['w1l_ap'][:], means[0][:, :C],
                                 start=True, stop=False)
                nc.tensor.matmul(psE[:, :C], wt['w1rb_ap'][:],
                                 pT65[:, c0:c0 + C], start=False, stop=True)
                nc.scalar.activation(p1t[0:64, :C], psE[:, :C], AF.Relu)
                psE2 = psE_p.tile([64, SB], f32, tag="psE")
                nc.tensor.matmul(psE2[:, :C], wt['w2l_ap'][:], means[1][:, :C],
                                 start=True, stop=False)
                nc.tensor.matmul(psE2[:, :C], wt['w2rb_ap'][:], p1t[:, :C],
                                 start=False, stop=True)
                p2 = mean_p.tile([64, SB], bf16)
                nc.scalar.activation(p2[:, :C], psE2[:, :C], AF.Relu)
                psH = psE_p.tile([64, SB], f32, tag="psE")
                nc.tensor.matmul(psH[:, :C], wt['wl1'][:], p2[:, :C],
                                 start=True, stop=True)
                h = mean_p.tile([64, SB], bf16)
                nc.scalar.activation(h[:, :C], psH[:, :C], AF.Relu,
                                     bias=wt['bl1c'][:])
                psO = psE_p.tile([1, SB], f32, tag="psO", bufs=1)
                nc.tensor.matmul(psO[:, :C], wt['wl2c'][:], h[:, :C],
                                 start=True, stop=True)
                o = sml_p.tile([1, SB], f32, tag="orow")
                nc.scalar.activation(o[:, :C], psO[:, :C], AF.Identity,
                                     bias=wt['bl2c'][:])
                nc.sync.dma_start(out_h[:, c0:c0 + C], o[:, :C])

            if STAGE >= 6:
                emit_pass(mAP, idxAP_h, ap_tabs, {2, 3}, epi_subB,
                          pre_acc=pre_subB)

    nc.compile()

    if int(os.environ.get("KERNEL_TLSIM", "1")):
        try:
            from concourse.timeline_sim import TimelineSim
            _t = TimelineSim(nc)
            kernel.modeled_time_ns = _t.simulate()
            print(f"[kernel] TimelineSim modeled core time: "
                  f"{kernel.modeled_time_ns / 1e3:.1f} us")
        except Exception as e:
            print(f"[kernel] TimelineSim failed: {e}")
            kernel.modeled_time_ns = None

    globals()["_last_nc"] = nc
    if int(os.environ.get("KERNEL_BUILD_ONLY", "0")):
        raise SystemExit(0)

    in_maps = []
    for c in range(N_CORES):
        m = {"afT": afT_cores[c], "p_rm": p_rm, "pT65": pT65_cores[c],
             "invA": invA_cores[c], "invP": invP_cores[c],
             "idxAP": idxAP[c], "idxPA": idxPA[c],
             "iota512": iota512, "ident128": ident128}
        for n, v in w_vals.items():
            m[n] = v
        for n, v in wf_vals.items():
            m[n] = v
        in_maps.append(m)

    if int(os.environ.get("KERNEL_SIM", "0")):
        from concourse import bass_interp
        sim = bass_interp.MultiCoreSim(nc, N_CORES)
        for c in range(N_CORES):
            for n, v in in_maps[c].items():
                sim.cores[c].tensor(n)[:] = v
        sim.simulate()
        results = [{"out": np.array(sim.cores[c].tensor("out"))}
                   for c in range(N_CORES)]
    else:
        trace = bool(int(os.environ.get("KERNEL_TRACE", "0")))
        res = run_bass_kernel_spmd(nc, in_maps, core_ids=list(range(N_CORES)),
                                   trace=trace)
        if trace:
            kernel.last_exec_time_ns = res.exec_time_ns
            kernel.last_results = res
        results = res.results

    out = np.concatenate([np.asarray(results[c]["out"], np.float32)[0]
                          for c in range(N_CORES)])[:NP_]
    return out.reshape(NP_, 1).astype(np.float32)
